# revision 13
# baseline (speedup 1.0000x reference)
"""MoE layer (B=2, S=2048, D=1024, E=8, K=2, H=4096) on 8 trn2 NeuronCores.

Strategy (expert-parallel):
  - Gating (x @ Wg, top-k, softmaxes, aux loss) runs on host via jax-CPU,
    bit-exact with the reference implementation.
  - Tokens are dispatched by selected expert: core e receives the tokens
    routed to expert e (padded to a common capacity C) plus expert e's
    weights, and computes gelu(x @ W1 + b1) @ W2 on device in fp16 with
    fp32 accumulation.
  - Host combines the per-assignment outputs with the top-k softmax
    weights (+ b2) and unpermutes back to [B, S, D].

Device kernel layout (all 8 cores run the same NEFF, SPMD):
  mm1: psum[h, tok] += W1[d, h].T @ xT[d, tok]   (W1 native [D,H] = lhsT)
  gelu+bias on ScalarE: hT tiles (PSUM f32 -> SBUF fp16)
  mm2: psum[tok, d] += hT[h, tok].T @ W2[h, d]   (W2 native [H,D] = rhs)
No transposes anywhere: x is pre-transposed on host; mm1 emits h^T.

Token blocks are processed mm1(b0), mm1(b1), mm2(b0), mm1(b2), mm2(b1),
... so the W2 stream (8.4 MB) has two mm1 phases (~80us) to arrive before
mm2 first reads it.  Input DMAs are batched into ~80 large transfers on a
single engine queue in strict need order (x^T, W1 chunk 0, W1 rest, W2):
each dma_start costs ~0.65us on the issuing sequencer, and parallel queues
make not-yet-needed streams steal HBM bandwidth from the critical one.
"""

import os
from contextlib import ExitStack

import numpy as np
import ml_dtypes

B, S, D, E, K = 2, 2048, 1024, 8, 2
H = 4 * D
N = B * S
P = 128

_last_results = None  # test harness introspection (BassKernelResults)

# device compute dtype for matmul operands (fp32 accumulation either way)
_DT_NAME = os.environ.get("MOE_DT", "float16")


def _block_sizes(C):
    """Token-block sizes for mm1 PSUM tiles.

    Capped at 384 (not the 512 PSUM-bank limit) so the double-buffered hT
    tile stays at 24 KB/partition per buffer and the whole working set fits
    in SBUF alongside the resident weights."""
    blocks = [384] * (C // 384)
    if C % 384:
        blocks.append(C % 384)
    return blocks


def _build_nc(C):
    import concourse.mybir as mybir
    import concourse.tile as tile
    from concourse import bacc

    dt = mybir.dt
    cdt = getattr(dt, _DT_NAME)
    n_d = D // P  # 8 contraction tiles for mm1
    n_h = H // P  # 32 contraction tiles for mm2
    D_BLK = 512
    n_db = D // D_BLK
    blocks = _block_sizes(C)
    nb = len(blocks)
    TB_MAX = max(blocks)
    # phase schedule: mm1 runs `lookahead` blocks ahead of mm2
    lookahead = min(2, nb)

    nc = bacc.Bacc("TRN2", target_bir_lowering=False, debug=False, num_devices=E)

    xt = nc.dram_tensor("xt", [n_d, P, C], cdt, kind="ExternalInput").ap()
    w1 = nc.dram_tensor("w1", [n_d, P, H], cdt, kind="ExternalInput").ap()
    w2 = nc.dram_tensor("w2", [n_h, P, D], cdt, kind="ExternalInput").ap()
    b1 = nc.dram_tensor("b1", [P, n_h], dt.float32, kind="ExternalInput").ap()
    y = nc.dram_tensor("y", [C, D], dt.float32, kind="ExternalOutput").ap()

    with tile.TileContext(nc) as tc, ExitStack() as ctx:
        wpool = ctx.enter_context(tc.tile_pool(name="wpool", bufs=1))
        hpool = ctx.enter_context(tc.tile_pool(name="hpool", bufs=lookahead))
        ps1 = ctx.enter_context(tc.tile_pool(name="ps1", bufs=4, space="PSUM"))
        ps2 = ctx.enter_context(tc.tile_pool(name="ps2", bufs=4, space="PSUM"))
        opool = ctx.enter_context(tc.tile_pool(name="opool", bufs=4))

        # ---- resident SBUF tensors (bufs=1 pools) ----
        w1_sb = wpool.tile([P, n_d, H], cdt, name="w1sb")  # 64 KB/part fp16
        w2_sb = wpool.tile([P, n_h, D], cdt, name="w2sb")  # 64 KB/part
        x_sb = wpool.tile([P, n_d, C], cdt, name="xsb")
        b1_sb = wpool.tile([P, n_h], dt.float32, name="b1sb")

        # ---- input DMAs: ONE engine (sync), strict need order ----
        # HBM serves in-flight transfers roughly fairly, so putting W2 on a
        # parallel queue makes it steal bandwidth from W1 ~80us before W2
        # is needed.  Strict ordering on one queue-set instead: x^T and the
        # first W1 chunk feed the first matmuls, later W1 chunks stay ahead
        # of mm1, W2 lands well before the first mm2 phase.
        # gpsimd: x^T (one plane per ~0.65us matches the PE's d-plane
        # consumption during the 4-group round-robin warmup), then y writes
        W1_CHUNK = 1024
        for d in range(n_d):
            nc.gpsimd.dma_start(x_sb[:, d, :], xt[d])
        # sync: W1 chunk 0 immediately, W1 rest, then W2 (strict need order)
        for hc in range(0, H, W1_CHUNK):
            for d in range(n_d):
                nc.sync.dma_start(
                    w1_sb[:, d, hc : hc + W1_CHUNK], w1[d, :, hc : hc + W1_CHUNK]
                )
        for ht in range(n_h):
            nc.sync.dma_start(w2_sb[:, ht, :], w2[ht])
        # scalar: b1 (host supplies it already transposed to [P, n_h])
        nc.scalar.dma_start(b1_sb[:], b1[:])

        # ---- compute phases ----
        h_tiles = [None] * nb
        t0s = np.concatenate([[0], np.cumsum(blocks)]).astype(int)

        def mm1(bi):
            tb = blocks[bi]
            t0 = t0s[bi]
            hT = hpool.tile([P, n_h, TB_MAX], cdt, name=f"hT{bi}", tag="hT")
            h_tiles[bi] = hT
            for ht in range(n_h):
                ps = ps1.tile([P, TB_MAX], dt.float32, name=f"ps1_{bi}_{ht}", tag="p1")
                for d in range(n_d):
                    nc.tensor.matmul(
                        ps[:, :tb],
                        w1_sb[:, d, ht * P : (ht + 1) * P],
                        x_sb[:, d, t0 : t0 + tb],
                        start=(d == 0),
                        stop=(d == n_d - 1),
                    )
                nc.scalar.activation(
                    hT[:, ht, :tb],
                    ps[:, :tb],
                    mybir.ActivationFunctionType.Gelu,
                    bias=b1_sb[:, ht : ht + 1],
                )

        def mm2(bi):
            tb = blocks[bi]
            t0 = t0s[bi]
            hT = h_tiles[bi]
            n_tt = (tb + P - 1) // P
            for db in range(n_db):
                for tt in range(n_tt):
                    m = min(P, tb - tt * P)
                    ps = ps2.tile(
                        [P, D_BLK], dt.float32, name=f"ps2_{bi}_{db}_{tt}", tag="p2"
                    )
                    for ht in range(n_h):
                        nc.tensor.matmul(
                            ps[:m, :],
                            hT[:, ht, tt * P : tt * P + m],
                            w2_sb[:, ht, db * D_BLK : (db + 1) * D_BLK],
                            start=(ht == 0),
                            stop=(ht == n_h - 1),
                        )
                    ot = opool.tile(
                        [P, D_BLK], dt.float32, name=f"ot{bi}_{db}_{tt}", tag="ot"
                    )
                    nc.vector.tensor_copy(ot[:m, :], ps[:m, :])
                    rows = t0 + tt * P
                    nc.gpsimd.dma_start(
                        y[rows : rows + m, db * D_BLK : (db + 1) * D_BLK], ot[:m, :]
                    )

        done1 = done2 = 0
        while done2 < nb:
            if done1 < min(done2 + lookahead, nb):
                mm1(done1)
                done1 += 1
            else:
                mm2(done2)
                done2 += 1

    nc.compile()
    return nc


def kernel(x, Wg, bg, W1, b1, W2, b2):
    global _last_results
    x = np.asarray(x)
    Wg = np.asarray(Wg)
    bg = np.asarray(bg)
    W1 = np.asarray(W1)
    b1 = np.asarray(b1)
    W2 = np.asarray(W2)
    b2 = np.asarray(b2)

    # ---- 1. gating on host, bit-exact with the reference (jax on CPU) ----
    import jax
    import jax.numpy as jnp

    cpu = jax.local_devices(backend="cpu")[0]
    with jax.default_device(cpu):
        xj = jnp.asarray(x)
        gate_logits = jnp.einsum("bsd,de->bse", xj, jnp.asarray(Wg)) + jnp.asarray(bg)
        probs = jax.nn.softmax(gate_logits, axis=-1)
        weights, selected_experts = jax.lax.top_k(gate_logits, K)
        topk_probs = jax.nn.softmax(weights, axis=-1)
        count_mask = jax.nn.one_hot(selected_experts, E, dtype=probs.dtype).sum(axis=2)
        me = probs.mean(axis=(0, 1))
        ce = count_mask.mean(axis=(0, 1))
        aux_loss = E * jnp.sum(me * ce)
        sel = np.asarray(selected_experts)  # [B,S,K] int32
        tp = np.asarray(topk_probs)  # [B,S,K] f32
        aux = np.asarray(aux_loss)

    # ---- 2. dispatch: sort assignments by expert ----
    x_flat = x.reshape(N, D)
    flat_e = sel.reshape(-1).astype(np.int64)  # [N*K]
    order = np.argsort(flat_e, kind="stable")
    counts = np.bincount(flat_e, minlength=E)
    starts = np.zeros(E, np.int64)
    np.cumsum(counts[:-1], out=starts[1:])
    C = max(512, int(counts.max()))

    token_of = np.arange(N * K) // K
    tok_sorted = token_of[order]

    n_d = D // P
    n_h = H // P
    cdt = getattr(ml_dtypes, _DT_NAME, None) or getattr(np, _DT_NAME)

    in_maps = []
    for e in range(E):
        toks = tok_sorted[starts[e] : starts[e] + counts[e]]
        xT = np.zeros((D, C), np.float32)
        xT[:, : counts[e]] = x_flat[toks].T
        in_maps.append(
            {
                "xt": np.ascontiguousarray(xT.reshape(n_d, P, C)).astype(cdt),
                "w1": np.ascontiguousarray(W1[e].reshape(n_d, P, H)).astype(cdt),
                "w2": np.ascontiguousarray(W2[e].reshape(n_h, P, D)).astype(cdt),
                "b1": np.ascontiguousarray(b1[e].reshape(n_h, P).T).astype(np.float32),
            }
        )

    # ---- 3. compile + run on 8 cores ----
    from concourse import bass_utils

    nc = _build_nc(C)
    try:
        res = bass_utils.run_bass_kernel_spmd(nc, in_maps, core_ids=list(range(E)))
    except Exception:
        # transient NRT device errors have been observed; retry once
        import time

        time.sleep(5)
        res = bass_utils.run_bass_kernel_spmd(nc, in_maps, core_ids=list(range(E)))
    _last_results = res
    Y = np.stack([r["y"] for r in res.results])  # [E, C, D] f32

    # ---- 4. combine ----
    slot = np.empty(N * K, np.int64)
    slot[order] = np.arange(N * K) - np.repeat(starts, counts)
    y_assign = Y[flat_e, slot]  # [N*K, D]
    out = (tp.reshape(N * K, 1) * y_assign).reshape(N, K, D).sum(axis=1)
    out = out + (tp.reshape(N, K, 1) * b2[sel.reshape(N, K)]).sum(axis=1)
    out = out.reshape(B, S, D).astype(np.float32)

    return out, sel, aux


# revision 15
# speedup vs baseline: 1.0081x; 1.0081x over previous
"""MoE layer (B=2, S=2048, D=1024, E=8, K=2, H=4096) on 8 trn2 NeuronCores.

Strategy (expert-parallel):
  - Gating (x @ Wg, top-k, softmaxes, aux loss) runs on host via jax-CPU,
    bit-exact with the reference implementation.
  - Tokens are dispatched by selected expert: core e receives the tokens
    routed to expert e (padded to a common capacity C) plus expert e's
    weights, and computes gelu(x @ W1 + b1) @ W2 on device in fp16 with
    fp32 accumulation.
  - Host combines the per-assignment outputs with the top-k softmax
    weights (+ b2) and unpermutes back to [B, S, D].

Device kernel layout (all 8 cores run the same NEFF, SPMD):
  mm1: psum[h, tok] += W1[d, h].T @ xT[d, tok]   (W1 native [D,H] = lhsT)
  gelu+bias on ScalarE: hT tiles (PSUM f32 -> SBUF fp16)
  mm2: psum[tok, d] += hT[h, tok].T @ W2[h, d]   (W2 native [H,D] = rhs)
No transposes anywhere: x is pre-transposed on host; mm1 emits h^T.

Token blocks are processed mm1(b0), mm1(b1), mm2(b0), mm1(b2), mm2(b1),
... so the W2 stream (8.4 MB) has two mm1 phases (~80us) to arrive before
mm2 first reads it.  Input DMAs are batched into ~80 large transfers on a
single engine queue in strict need order (x^T, W1 chunk 0, W1 rest, W2):
each dma_start costs ~0.65us on the issuing sequencer, and parallel queues
make not-yet-needed streams steal HBM bandwidth from the critical one.
"""

import os
from contextlib import ExitStack

import numpy as np
import ml_dtypes

B, S, D, E, K = 2, 2048, 1024, 8, 2
H = 4 * D
N = B * S
P = 128

_last_results = None  # test harness introspection (BassKernelResults)

# device compute dtype for matmul operands (fp32 accumulation either way)
_DT_NAME = os.environ.get("MOE_DT", "float16")


def _block_sizes(C):
    """Token-block sizes for mm1 PSUM tiles.

    Capped at 384 (not the 512 PSUM-bank limit) so the double-buffered hT
    tile stays at 24 KB/partition per buffer and the whole working set fits
    in SBUF alongside the resident weights."""
    blocks = [384] * (C // 384)
    if C % 384:
        blocks.append(C % 384)
    return blocks


def _build_nc(C):
    import concourse.mybir as mybir
    import concourse.tile as tile
    from concourse import bacc

    dt = mybir.dt
    cdt = getattr(dt, _DT_NAME)
    n_d = D // P  # 8 contraction tiles for mm1
    n_h = H // P  # 32 contraction tiles for mm2
    D_BLK = 512
    n_db = D // D_BLK
    blocks = _block_sizes(C)
    nb = len(blocks)
    TB_MAX = max(blocks)
    # phase schedule: mm1 runs `lookahead` blocks ahead of mm2
    lookahead = min(2, nb)

    nc = bacc.Bacc("TRN2", target_bir_lowering=False, debug=False, num_devices=E)

    xt = nc.dram_tensor("xt", [n_d, P, C], cdt, kind="ExternalInput").ap()
    w1 = nc.dram_tensor("w1", [n_d, P, H], cdt, kind="ExternalInput").ap()
    w2 = nc.dram_tensor("w2", [n_h, P, D], cdt, kind="ExternalInput").ap()
    b1 = nc.dram_tensor("b1", [P, n_h], dt.float32, kind="ExternalInput").ap()
    y = nc.dram_tensor("y", [C, D], dt.float32, kind="ExternalOutput").ap()

    with tile.TileContext(nc) as tc, ExitStack() as ctx:
        wpool = ctx.enter_context(tc.tile_pool(name="wpool", bufs=1))
        hpool = ctx.enter_context(tc.tile_pool(name="hpool", bufs=lookahead))
        ps1 = ctx.enter_context(tc.tile_pool(name="ps1", bufs=4, space="PSUM"))
        ps2 = ctx.enter_context(tc.tile_pool(name="ps2", bufs=4, space="PSUM"))
        opool = ctx.enter_context(tc.tile_pool(name="opool", bufs=4))

        # ---- resident SBUF tensors (bufs=1 pools) ----
        w1_sb = wpool.tile([P, n_d, H], cdt, name="w1sb")  # 64 KB/part fp16
        w2_sb = wpool.tile([P, n_h, D], cdt, name="w2sb")  # 64 KB/part
        x_sb = wpool.tile([P, n_d, C], cdt, name="xsb")
        b1_sb = wpool.tile([P, n_h], dt.float32, name="b1sb")

        # ---- input DMAs: ONE engine (sync), strict need order ----
        # HBM serves in-flight transfers roughly fairly, so putting W2 on a
        # parallel queue makes it steal bandwidth from W1 ~80us before W2
        # is needed.  Strict ordering on one queue-set instead: x^T and the
        # first W1 chunk feed the first matmuls, later W1 chunks stay ahead
        # of mm1, W2 lands well before the first mm2 phase.
        W1_CHUNK = 1024
        for d in range(n_d):
            nc.sync.dma_start(x_sb[:, d, :], xt[d])
            nc.sync.dma_start(w1_sb[:, d, 0:W1_CHUNK], w1[d, :, 0:W1_CHUNK])
        for hc in range(W1_CHUNK, H, W1_CHUNK):
            for d in range(n_d):
                nc.sync.dma_start(
                    w1_sb[:, d, hc : hc + W1_CHUNK], w1[d, :, hc : hc + W1_CHUNK]
                )
        for ht in range(n_h):
            nc.sync.dma_start(w2_sb[:, ht, :], w2[ht])
        # scalar: b1 (host supplies it already transposed to [P, n_h])
        nc.scalar.dma_start(b1_sb[:], b1[:])

        # ---- compute phases ----
        h_tiles = [None] * nb
        t0s = np.concatenate([[0], np.cumsum(blocks)]).astype(int)

        def mm1(bi):
            tb = blocks[bi]
            t0 = t0s[bi]
            hT = hpool.tile([P, n_h, TB_MAX], cdt, name=f"hT{bi}", tag="hT")
            h_tiles[bi] = hT
            for ht in range(n_h):
                ps = ps1.tile([P, TB_MAX], dt.float32, name=f"ps1_{bi}_{ht}", tag="p1")
                for d in range(n_d):
                    nc.tensor.matmul(
                        ps[:, :tb],
                        w1_sb[:, d, ht * P : (ht + 1) * P],
                        x_sb[:, d, t0 : t0 + tb],
                        start=(d == 0),
                        stop=(d == n_d - 1),
                    )
                nc.scalar.activation(
                    hT[:, ht, :tb],
                    ps[:, :tb],
                    mybir.ActivationFunctionType.Gelu,
                    bias=b1_sb[:, ht : ht + 1],
                )

        def mm2(bi):
            tb = blocks[bi]
            t0 = t0s[bi]
            hT = h_tiles[bi]
            n_tt = (tb + P - 1) // P
            for db in range(n_db):
                for tt in range(n_tt):
                    m = min(P, tb - tt * P)
                    ps = ps2.tile(
                        [P, D_BLK], dt.float32, name=f"ps2_{bi}_{db}_{tt}", tag="p2"
                    )
                    for ht in range(n_h):
                        nc.tensor.matmul(
                            ps[:m, :],
                            hT[:, ht, tt * P : tt * P + m],
                            w2_sb[:, ht, db * D_BLK : (db + 1) * D_BLK],
                            start=(ht == 0),
                            stop=(ht == n_h - 1),
                        )
                    ot = opool.tile(
                        [P, D_BLK], dt.float32, name=f"ot{bi}_{db}_{tt}", tag="ot"
                    )
                    nc.vector.tensor_copy(ot[:m, :], ps[:m, :])
                    rows = t0 + tt * P
                    nc.sync.dma_start(
                        y[rows : rows + m, db * D_BLK : (db + 1) * D_BLK], ot[:m, :]
                    )

        done1 = done2 = 0
        while done2 < nb:
            if done1 < min(done2 + lookahead, nb):
                mm1(done1)
                done1 += 1
            else:
                mm2(done2)
                done2 += 1

    nc.compile()
    return nc


def kernel(x, Wg, bg, W1, b1, W2, b2):
    global _last_results
    x = np.asarray(x)
    Wg = np.asarray(Wg)
    bg = np.asarray(bg)
    W1 = np.asarray(W1)
    b1 = np.asarray(b1)
    W2 = np.asarray(W2)
    b2 = np.asarray(b2)

    # ---- 1. gating on host, bit-exact with the reference (jax on CPU) ----
    import jax
    import jax.numpy as jnp

    cpu = jax.local_devices(backend="cpu")[0]
    with jax.default_device(cpu):
        xj = jnp.asarray(x)
        gate_logits = jnp.einsum("bsd,de->bse", xj, jnp.asarray(Wg)) + jnp.asarray(bg)
        probs = jax.nn.softmax(gate_logits, axis=-1)
        weights, selected_experts = jax.lax.top_k(gate_logits, K)
        topk_probs = jax.nn.softmax(weights, axis=-1)
        count_mask = jax.nn.one_hot(selected_experts, E, dtype=probs.dtype).sum(axis=2)
        me = probs.mean(axis=(0, 1))
        ce = count_mask.mean(axis=(0, 1))
        aux_loss = E * jnp.sum(me * ce)
        sel = np.asarray(selected_experts)  # [B,S,K] int32
        tp = np.asarray(topk_probs)  # [B,S,K] f32
        aux = np.asarray(aux_loss)

    # ---- 2. dispatch: sort assignments by expert ----
    x_flat = x.reshape(N, D)
    flat_e = sel.reshape(-1).astype(np.int64)  # [N*K]
    order = np.argsort(flat_e, kind="stable")
    counts = np.bincount(flat_e, minlength=E)
    starts = np.zeros(E, np.int64)
    np.cumsum(counts[:-1], out=starts[1:])
    C = max(512, int(counts.max()))

    token_of = np.arange(N * K) // K
    tok_sorted = token_of[order]

    n_d = D // P
    n_h = H // P
    cdt = getattr(ml_dtypes, _DT_NAME, None) or getattr(np, _DT_NAME)

    in_maps = []
    for e in range(E):
        toks = tok_sorted[starts[e] : starts[e] + counts[e]]
        xT = np.zeros((D, C), np.float32)
        xT[:, : counts[e]] = x_flat[toks].T
        in_maps.append(
            {
                "xt": np.ascontiguousarray(xT.reshape(n_d, P, C)).astype(cdt),
                "w1": np.ascontiguousarray(W1[e].reshape(n_d, P, H)).astype(cdt),
                "w2": np.ascontiguousarray(W2[e].reshape(n_h, P, D)).astype(cdt),
                "b1": np.ascontiguousarray(b1[e].reshape(n_h, P).T).astype(np.float32),
            }
        )

    # ---- 3. compile + run on 8 cores ----
    from concourse import bass_utils

    nc = _build_nc(C)
    try:
        res = bass_utils.run_bass_kernel_spmd(nc, in_maps, core_ids=list(range(E)))
    except Exception:
        # transient NRT device errors have been observed; retry once
        import time

        time.sleep(5)
        res = bass_utils.run_bass_kernel_spmd(nc, in_maps, core_ids=list(range(E)))
    _last_results = res
    Y = np.stack([r["y"] for r in res.results])  # [E, C, D] f32

    # ---- 4. combine ----
    slot = np.empty(N * K, np.int64)
    slot[order] = np.arange(N * K) - np.repeat(starts, counts)
    y_assign = Y[flat_e, slot]  # [N*K, D]
    out = (tp.reshape(N * K, 1) * y_assign).reshape(N, K, D).sum(axis=1)
    out = out + (tp.reshape(N, K, 1) * b2[sel.reshape(N, K)]).sum(axis=1)
    out = out.reshape(B, S, D).astype(np.float32)

    return out, sel, aux


# revision 16
# speedup vs baseline: 1.0155x; 1.0074x over previous
"""MoE layer (B=2, S=2048, D=1024, E=8, K=2, H=4096) on 8 trn2 NeuronCores.

Strategy (expert-parallel):
  - Gating (x @ Wg, top-k, softmaxes, aux loss) runs on host via jax-CPU,
    bit-exact with the reference implementation.
  - Tokens are dispatched by selected expert: core e receives the tokens
    routed to expert e (padded to a common capacity C) plus expert e's
    weights, and computes gelu(x @ W1 + b1) @ W2 on device in fp16 with
    fp32 accumulation.
  - Host combines the per-assignment outputs with the top-k softmax
    weights (+ b2) and unpermutes back to [B, S, D].

Device kernel layout (all 8 cores run the same NEFF, SPMD):
  mm1: psum[h, tok] += W1[d, h].T @ xT[d, tok]   (W1 native [D,H] = lhsT)
  gelu+bias on ScalarE: hT tiles (PSUM f32 -> SBUF fp16)
  mm2: psum[tok, d] += hT[h, tok].T @ W2[h, d]   (W2 native [H,D] = rhs)
No transposes anywhere: x is pre-transposed on host; mm1 emits h^T.

Token blocks are processed mm1(b0), mm1(b1), mm2(b0), mm1(b2), mm2(b1),
... so the W2 stream (8.4 MB) has two mm1 phases (~80us) to arrive before
mm2 first reads it.  Input DMAs are batched into ~80 large transfers on a
single engine queue in strict need order (x^T, W1 chunk 0, W1 rest, W2):
each dma_start costs ~0.65us on the issuing sequencer, and parallel queues
make not-yet-needed streams steal HBM bandwidth from the critical one.
"""

import os
from contextlib import ExitStack

import numpy as np
import ml_dtypes

B, S, D, E, K = 2, 2048, 1024, 8, 2
H = 4 * D
N = B * S
P = 128

_last_results = None  # test harness introspection (BassKernelResults)

# device compute dtype for matmul operands (fp32 accumulation either way)
_DT_NAME = os.environ.get("MOE_DT", "float16")


def _block_sizes(C):
    """Token-block sizes for mm1 PSUM tiles.

    Capped at 384 (not the 512 PSUM-bank limit) so the double-buffered hT
    tile stays at 24 KB/partition per buffer and the whole working set fits
    in SBUF alongside the resident weights."""
    blocks = [384] * (C // 384)
    if C % 384:
        blocks.append(C % 384)
    return blocks


def _build_nc(C):
    import concourse.mybir as mybir
    import concourse.tile as tile
    from concourse import bacc

    dt = mybir.dt
    cdt = getattr(dt, _DT_NAME)
    n_d = D // P  # 8 contraction tiles for mm1
    n_h = H // P  # 32 contraction tiles for mm2
    D_BLK = 512
    n_db = D // D_BLK
    blocks = _block_sizes(C)
    nb = len(blocks)
    TB_MAX = max(blocks)
    # phase schedule: mm1 runs `lookahead` blocks ahead of mm2
    lookahead = min(2, nb)

    nc = bacc.Bacc("TRN2", target_bir_lowering=False, debug=False, num_devices=E)

    xt = nc.dram_tensor("xt", [n_d, P, C], cdt, kind="ExternalInput").ap()
    w1 = nc.dram_tensor("w1", [n_d, P, H], cdt, kind="ExternalInput").ap()
    w2 = nc.dram_tensor("w2", [n_h, P, D], cdt, kind="ExternalInput").ap()
    b1 = nc.dram_tensor("b1", [P, n_h], dt.float32, kind="ExternalInput").ap()
    y = nc.dram_tensor("y", [C, D], dt.float32, kind="ExternalOutput").ap()

    with tile.TileContext(nc) as tc, ExitStack() as ctx:
        wpool = ctx.enter_context(tc.tile_pool(name="wpool", bufs=1))
        hpool = ctx.enter_context(tc.tile_pool(name="hpool", bufs=lookahead))
        ps1 = ctx.enter_context(tc.tile_pool(name="ps1", bufs=5, space="PSUM"))
        ps2 = ctx.enter_context(tc.tile_pool(name="ps2", bufs=3, space="PSUM"))
        opool = ctx.enter_context(tc.tile_pool(name="opool", bufs=4))

        # ---- resident SBUF tensors (bufs=1 pools) ----
        w1_sb = wpool.tile([P, n_d, H], cdt, name="w1sb")  # 64 KB/part fp16
        w2_sb = wpool.tile([P, n_h, D], cdt, name="w2sb")  # 64 KB/part
        x_sb = wpool.tile([P, n_d, C], cdt, name="xsb")
        b1_sb = wpool.tile([P, n_h], dt.float32, name="b1sb")

        # ---- input DMAs: ONE engine (sync), strict need order ----
        # HBM serves in-flight transfers roughly fairly, so putting W2 on a
        # parallel queue makes it steal bandwidth from W1 ~80us before W2
        # is needed.  Strict ordering on one queue-set instead: x^T and the
        # first W1 chunk feed the first matmuls, later W1 chunks stay ahead
        # of mm1, W2 lands well before the first mm2 phase.
        W1_CHUNK = 1024
        # x^T issues from gpsimd in parallel with W1 on sync: one d-plane
        # pair per ~0.65us matches the PE's 4-group round-robin consumption
        # during warmup (single-queue cadence of 1.3us/pair made PE crawl)
        for d in range(n_d):
            nc.gpsimd.dma_start(x_sb[:, d, :], xt[d])
        for hc in range(0, H, W1_CHUNK):
            for d in range(n_d):
                nc.sync.dma_start(
                    w1_sb[:, d, hc : hc + W1_CHUNK], w1[d, :, hc : hc + W1_CHUNK]
                )
        for ht in range(n_h):
            nc.sync.dma_start(w2_sb[:, ht, :], w2[ht])
        # scalar: b1 (host supplies it already transposed to [P, n_h])
        nc.scalar.dma_start(b1_sb[:], b1[:])

        # ---- compute phases ----
        h_tiles = [None] * nb
        t0s = np.concatenate([[0], np.cumsum(blocks)]).astype(int)

        def mm1(bi):
            tb = blocks[bi]
            t0 = t0s[bi]
            hT = hpool.tile([P, n_h, TB_MAX], cdt, name=f"hT{bi}", tag="hT")
            h_tiles[bi] = hT
            for ht in range(n_h):
                ps = ps1.tile([P, TB_MAX], dt.float32, name=f"ps1_{bi}_{ht}", tag="p1")
                for d in range(n_d):
                    nc.tensor.matmul(
                        ps[:, :tb],
                        w1_sb[:, d, ht * P : (ht + 1) * P],
                        x_sb[:, d, t0 : t0 + tb],
                        start=(d == 0),
                        stop=(d == n_d - 1),
                    )
                nc.scalar.activation(
                    hT[:, ht, :tb],
                    ps[:, :tb],
                    mybir.ActivationFunctionType.Gelu,
                    bias=b1_sb[:, ht : ht + 1],
                )

        def mm2(bi):
            tb = blocks[bi]
            t0 = t0s[bi]
            hT = h_tiles[bi]
            n_tt = (tb + P - 1) // P
            for db in range(n_db):
                for tt in range(n_tt):
                    m = min(P, tb - tt * P)
                    ps = ps2.tile(
                        [P, D_BLK], dt.float32, name=f"ps2_{bi}_{db}_{tt}", tag="p2"
                    )
                    for ht in range(n_h):
                        nc.tensor.matmul(
                            ps[:m, :],
                            hT[:, ht, tt * P : tt * P + m],
                            w2_sb[:, ht, db * D_BLK : (db + 1) * D_BLK],
                            start=(ht == 0),
                            stop=(ht == n_h - 1),
                        )
                    ot = opool.tile(
                        [P, D_BLK], dt.float32, name=f"ot{bi}_{db}_{tt}", tag="ot"
                    )
                    nc.vector.tensor_copy(ot[:m, :], ps[:m, :])
                    rows = t0 + tt * P
                    nc.sync.dma_start(
                        y[rows : rows + m, db * D_BLK : (db + 1) * D_BLK], ot[:m, :]
                    )

        done1 = done2 = 0
        while done2 < nb:
            if done1 < min(done2 + lookahead, nb):
                mm1(done1)
                done1 += 1
            else:
                mm2(done2)
                done2 += 1

    nc.compile()
    return nc


def kernel(x, Wg, bg, W1, b1, W2, b2):
    global _last_results
    x = np.asarray(x)
    Wg = np.asarray(Wg)
    bg = np.asarray(bg)
    W1 = np.asarray(W1)
    b1 = np.asarray(b1)
    W2 = np.asarray(W2)
    b2 = np.asarray(b2)

    # ---- 1. gating on host, bit-exact with the reference (jax on CPU) ----
    import jax
    import jax.numpy as jnp

    cpu = jax.local_devices(backend="cpu")[0]
    with jax.default_device(cpu):
        xj = jnp.asarray(x)
        gate_logits = jnp.einsum("bsd,de->bse", xj, jnp.asarray(Wg)) + jnp.asarray(bg)
        probs = jax.nn.softmax(gate_logits, axis=-1)
        weights, selected_experts = jax.lax.top_k(gate_logits, K)
        topk_probs = jax.nn.softmax(weights, axis=-1)
        count_mask = jax.nn.one_hot(selected_experts, E, dtype=probs.dtype).sum(axis=2)
        me = probs.mean(axis=(0, 1))
        ce = count_mask.mean(axis=(0, 1))
        aux_loss = E * jnp.sum(me * ce)
        sel = np.asarray(selected_experts)  # [B,S,K] int32
        tp = np.asarray(topk_probs)  # [B,S,K] f32
        aux = np.asarray(aux_loss)

    # ---- 2. dispatch: sort assignments by expert ----
    x_flat = x.reshape(N, D)
    flat_e = sel.reshape(-1).astype(np.int64)  # [N*K]
    order = np.argsort(flat_e, kind="stable")
    counts = np.bincount(flat_e, minlength=E)
    starts = np.zeros(E, np.int64)
    np.cumsum(counts[:-1], out=starts[1:])
    C = max(512, int(counts.max()))

    token_of = np.arange(N * K) // K
    tok_sorted = token_of[order]

    n_d = D // P
    n_h = H // P
    cdt = getattr(ml_dtypes, _DT_NAME, None) or getattr(np, _DT_NAME)

    in_maps = []
    for e in range(E):
        toks = tok_sorted[starts[e] : starts[e] + counts[e]]
        xT = np.zeros((D, C), np.float32)
        xT[:, : counts[e]] = x_flat[toks].T
        in_maps.append(
            {
                "xt": np.ascontiguousarray(xT.reshape(n_d, P, C)).astype(cdt),
                "w1": np.ascontiguousarray(W1[e].reshape(n_d, P, H)).astype(cdt),
                "w2": np.ascontiguousarray(W2[e].reshape(n_h, P, D)).astype(cdt),
                "b1": np.ascontiguousarray(b1[e].reshape(n_h, P).T).astype(np.float32),
            }
        )

    # ---- 3. compile + run on 8 cores ----
    from concourse import bass_utils

    nc = _build_nc(C)
    try:
        res = bass_utils.run_bass_kernel_spmd(nc, in_maps, core_ids=list(range(E)))
    except Exception:
        # transient NRT device errors have been observed; retry once
        import time

        time.sleep(5)
        res = bass_utils.run_bass_kernel_spmd(nc, in_maps, core_ids=list(range(E)))
    _last_results = res
    Y = np.stack([r["y"] for r in res.results])  # [E, C, D] f32

    # ---- 4. combine ----
    slot = np.empty(N * K, np.int64)
    slot[order] = np.arange(N * K) - np.repeat(starts, counts)
    y_assign = Y[flat_e, slot]  # [N*K, D]
    out = (tp.reshape(N * K, 1) * y_assign).reshape(N, K, D).sum(axis=1)
    out = out + (tp.reshape(N, K, 1) * b2[sel.reshape(N, K)]).sum(axis=1)
    out = out.reshape(B, S, D).astype(np.float32)

    return out, sel, aux
